# revision 1
# baseline (speedup 1.0000x reference)
"""Depthwise 2D cross-correlation on TRN2 via FFT-matmul pipeline (v3).

Per channel (64x64 image x, 16x16 template t, 49x49 valid output):
  F1 : X1[97,64] = B1[97,64] @ x       rows: Re f 0..32 | zeros | Im f 64..96
  T1 : XT[64,97] = X1^T                one PE transpose (zero band rides along)
  F2 : X2[128,33] = l2a^T @ XT[:,Re] + l2b^T @ XT[:,Im]   (psum accumulate)
       partitions of X2: Re g 0..63 | Im g 64..127
  PW : P1 = X2 * T2,  P2 = X2 * T2swz  (DVE; T2swz = halves-swapped spectrum,
       produced by a second F2t matmul pass with column-permuted bases)
  I1 : V[113,33] = lp1^T @ P1 + lp2^T @ P2   rows: Re j 0..48 | zeros | Im j 64..112
  TI : VT[33,113] = V^T                one PE transpose
  I2 : out[49,49] = li2a^T @ VT[:,Re] + li2b^T @ VT[:,Im]
All engine partition bases are 0 or 64; transposes write PSUM base 0 only.
"""
import sys
sys.path.insert(0, '/opt/trn_rl_repo')
import numpy as np

import concourse.tile_sem_assignment as tsa
# walrus in this container only supports ONE sync-wait on the tail drain:
# route every DMA through a single HWDGE queue sem.
tsa.NUM_HWDGE_SEMS = 1
tsa.NUM_SWDGE_GLOBAL_SEMS = 1

import concourse.bass as bass
from concourse import bacc
import concourse.tile as tile
from concourse import mybir

F32 = mybir.dt.float32
N = 64
NF = 33          # f halfplane 0..32
W1 = 97          # Re 0..32 | zero band | Im 64..96
HO = 49
WV = 113         # Re j 0..48 | zero band | Im j 64..112
HT = 16


def make_bases(dtype=np.float32):
    f = np.arange(NF)[:, None]
    r = np.arange(N)[None, :]
    ang = 2 * np.pi * f * r / N
    B1 = np.zeros((W1, N))
    B1[0:NF] = np.cos(ang)
    B1[64:W1] = -np.sin(ang)

    g = np.arange(N)[None, :]
    col = np.arange(N)[:, None]
    a2 = 2 * np.pi * g * col / N
    c2, s2 = np.cos(a2), np.sin(a2)
    # F2 split lhsT: contraction over col; M = (Re g | Im g)
    l2a = np.concatenate([c2, -s2], axis=1)    # [64, 128] for ReX1 slots
    l2b = np.concatenate([s2, c2], axis=1)     # [64, 128] for ImX1 slots
    perm = np.concatenate([np.arange(64, 128), np.arange(64)])

    gg = np.arange(N)[:, None]
    j = np.arange(HO)[None, :]
    a = 2 * np.pi * gg * j / N
    c, s = np.cos(a), np.sin(a)
    lhsT_P1 = np.zeros((128, WV))
    lhsT_P1[:64, :HO] = c
    lhsT_P1[64:, :HO] = c
    lhsT_P1[:64, 64:] = s
    lhsT_P1[64:, 64:] = s
    lhsT_P2 = np.zeros((128, WV))
    lhsT_P2[:64, :HO] = s
    lhsT_P2[64:, :HO] = -s
    lhsT_P2[:64, 64:] = -c
    lhsT_P2[64:, 64:] = c

    ff = np.arange(NF)[:, None]
    i = np.arange(HO)[None, :]
    af = 2 * np.pi * ff * i / N
    cf = np.where((ff >= 1) & (ff <= 31), 2.0, 1.0) / (N * N)
    li2a = cf * np.cos(af)                     # [33, 49]
    li2b = -cf * np.sin(af)                    # [33, 49]

    return {
        "b1": np.ascontiguousarray(B1.T, dtype),            # [64, 97]
        "b1t": np.ascontiguousarray(B1[:, :HT].T, dtype),   # [16, 97]
        "l2a": np.ascontiguousarray(l2a, dtype),            # [64, 128]
        "l2b": np.ascontiguousarray(l2b, dtype),            # [64, 128]
        "l2ta": np.ascontiguousarray(l2a[:HT], dtype),      # [16, 128]
        "l2tb": np.ascontiguousarray(l2b[:HT], dtype),      # [16, 128]
        "l2tas": np.ascontiguousarray(l2a[:HT][:, perm], dtype),
        "l2tbs": np.ascontiguousarray(l2b[:HT][:, perm], dtype),
        "lp1": np.ascontiguousarray(lhsT_P1, dtype),        # [128, 113]
        "lp2": np.ascontiguousarray(lhsT_P2, dtype),        # [128, 113]
        "li2a": np.ascontiguousarray(li2a, dtype),          # [33, 49]
        "li2b": np.ascontiguousarray(li2b, dtype),          # [33, 49]
        "ident": np.ascontiguousarray(np.eye(128), dtype),  # [128, 128]
    }


KEYS = ("b1", "b1t", "l2a", "l2b", "l2ta", "l2tb", "l2tas", "l2tbs",
        "lp1", "lp2", "li2a", "li2b", "ident")


def build(nch, chunk=64, rep=1):
    """Build the Bass program for `nch` channels per core."""
    assert nch % chunk == 0 and chunk % 8 == 0
    nchunks = nch // chunk
    ngrp = chunk // 8              # groups of 8 channels

    nc = bacc.Bacc()
    x_d = nc.declare_dram_parameter("x", [nch, N, N], F32, isOutput=False)
    t_d = nc.declare_dram_parameter("t", [nch, HT, HT], F32, isOutput=False)
    o_d = nc.declare_dram_parameter("o", [nch, HO, HO], F32, isOutput=True)
    basnp = make_bases()
    bas = {k: nc.declare_dram_parameter(k, list(basnp[k].shape), F32,
                                        isOutput=False) for k in KEYS}

    with tile.TileContext(nc) as tc:
        with (
            tc.tile_pool(name="const", bufs=1) as cpool,
            tc.tile_pool(name="xin", bufs=3) as xin,
            tc.tile_pool(name="sb1", bufs=3) as sb1,
            tc.tile_pool(name="sb2", bufs=3) as sb2,
            tc.tile_pool(name="sb3", bufs=3) as sb3,
            tc.tile_pool(name="sbo", bufs=3) as sbo,
            tc.tile_pool(name="psA", bufs=1, space="PSUM") as psA,
            tc.tile_pool(name="psT", bufs=3, space="PSUM") as psT,
            tc.tile_pool(name="psM", bufs=2, space="PSUM") as psM,
            tc.tile_pool(name="psV", bufs=1, space="PSUM") as psV,
            tc.tile_pool(name="psO", bufs=1, space="PSUM") as psO,
        ):
            cb = {}
            for k in KEYS:
                cb[k] = cpool.tile(list(basnp[k].shape), F32, tag=k, name=k)
                nc.sync.dma_start(cb[k][:], bas[k][:])
            idn = cb["ident"]

            for _rep in range(rep):
              for ci in range(nchunks):
                c0 = ci * chunk
                # ---------------- t path ----------------
                tt = xin.tile([HT, chunk, HT], F32, tag="tt")
                nc.sync.dma_start(tt[:], t_d[c0:c0 + chunk].transpose([1, 0, 2]))
                ttf = tt.rearrange("a b c -> a (b c)")
                x1t = sb1.tile([W1, chunk * HT], F32, tag="x1t")
                seg = min(512, chunk * HT)
                for h in range(chunk * HT // seg):
                    pt = psA.tile([W1, seg], F32, tag="pA", name="pt")
                    sl = slice(h * seg, (h + 1) * seg)
                    nc.tensor.matmul(pt[:], cb["b1t"][:], ttf[:, sl])
                    nc.scalar.copy(x1t[:, sl], pt[:])
                t2sb = sb2.tile([128, chunk * NF], F32, tag="t2sb")
                for g8 in range(ngrp):
                    xtt = sb1.tile([HT, 8 * W1], F32, tag="xtt")
                    for k in range(8):
                        ch = g8 * 8 + k
                        ptt = psT.tile([HT, W1], F32, tag="pT", name="ptt")
                        nc.tensor.transpose(
                            ptt[:], x1t[:, ch * HT:(ch + 1) * HT],
                            idn[0:W1, 0:W1])
                        nc.vector.tensor_copy(
                            xtt[:, k * W1:(k + 1) * W1], ptt[:])
                    pt2 = psM.tile([128, 8 * NF], F32, tag="pM", name="pt2")
                    for k in range(8):
                        ksl = slice(k * NF, (k + 1) * NF)
                        re = slice(k * W1, k * W1 + NF)
                        nc.tensor.matmul(pt2[:, ksl], cb["l2ta"][:], xtt[:, re],
                                         start=(k == 0), stop=False)
                    for k in range(8):
                        ksl = slice(k * NF, (k + 1) * NF)
                        im = slice(k * W1 + 64, k * W1 + W1)
                        nc.tensor.matmul(pt2[:, ksl], cb["l2tb"][:], xtt[:, im],
                                         start=False, stop=(k == 7))
                    gsl = slice(g8 * 8 * NF, (g8 * 8 + 8) * NF)
                    nc.scalar.copy(t2sb[:, gsl], pt2[:])

                # ---------------- x path ----------------
                for g8 in range(ngrp):
                    ch0 = g8 * 8
                    xr = xin.tile([N, 8, N], F32, tag="xr")
                    nc.sync.dma_start(
                        xr[:], x_d[c0 + ch0:c0 + ch0 + 8].transpose([1, 0, 2]))
                    pf = psA.tile([W1, 512], F32, tag="pA", name="pf")
                    nc.tensor.matmul(pf[:], cb["b1"][:],
                                     xr.rearrange("a b c -> a (b c)")[:])
                    x1 = sb1.tile([W1, 512], F32, tag="x1")
                    nc.scalar.copy(x1[:], pf[:])
                    xt = sb2.tile([N, 8 * W1], F32, tag="xt")
                    for k in range(8):
                        pxt = psT.tile([N, W1], F32, tag="pT", name="pxt")
                        nc.tensor.transpose(
                            pxt[:], x1[:, k * N:(k + 1) * N], idn[0:W1, 0:W1])
                        nc.vector.tensor_copy(xt[:, k * W1:(k + 1) * W1],
                                              pxt[:])
                    px2 = psM.tile([128, 8 * NF], F32, tag="pM", name="px2")
                    for k in range(8):
                        ksl = slice(k * NF, (k + 1) * NF)
                        re = slice(k * W1, k * W1 + NF)
                        nc.tensor.matmul(px2[:, ksl], cb["l2a"][:], xt[:, re],
                                         start=(k == 0), stop=False)
                    for k in range(8):
                        ksl = slice(k * NF, (k + 1) * NF)
                        im = slice(k * W1 + 64, k * W1 + W1)
                        nc.tensor.matmul(px2[:, ksl], cb["l2b"][:], xt[:, im],
                                         start=False, stop=(k == 7))
                    # products (psum x sbuf -> sbuf)
                    gsl = slice(ch0 * NF, (ch0 + 8) * NF)
                    p1 = sb3.tile([128, 8 * NF], F32, tag="p1")
                    p2 = sb3.tile([128, 8 * NF], F32, tag="p2")
                    nc.vector.tensor_mul(p1[:], px2[:], t2sb[:, gsl])
                    gs0 = slice(ch0 * NF, (ch0 + 8) * NF)
                    t2top = t2sb[0:64, gs0]
                    t2bot = t2sb[64:128, gs0]
                    nc.vector.tensor_mul(p2[0:64, :], px2[0:64, :], t2bot)
                    nc.vector.tensor_mul(p2[64:128, :], px2[64:128, :], t2top)
                    # I1
                    pv = psV.tile([WV, 8 * NF], F32, tag="pV", name="pv")
                    for k in range(8):
                        ksl = slice(k * NF, (k + 1) * NF)
                        nc.tensor.matmul(pv[:, ksl], cb["lp1"][:], p1[:, ksl],
                                         start=(k == 0), stop=False)
                    for k in range(8):
                        ksl = slice(k * NF, (k + 1) * NF)
                        nc.tensor.matmul(pv[:, ksl], cb["lp2"][:], p2[:, ksl],
                                         start=False, stop=(k == 7))
                    vsb = sb2.tile([WV, 8 * NF], F32, tag="vsb")
                    nc.scalar.copy(vsb[:], pv[:])
                    # TI
                    vt = sb3.tile([NF, 8 * WV], F32, tag="vt")
                    for k in range(8):
                        pvt = psT.tile([NF, WV], F32, tag="pT", name="pvt")
                        nc.tensor.transpose(
                            pvt[:], vsb[:, k * NF:(k + 1) * NF],
                            idn[0:WV, 0:WV])
                        nc.vector.tensor_copy(vt[:, k * WV:(k + 1) * WV],
                                              pvt[:])
                    # I2
                    po = psO.tile([HO, 8 * HO], F32, tag="pO", name="po")
                    for k in range(8):
                        osl = slice(k * HO, (k + 1) * HO)
                        re = slice(k * WV, k * WV + HO)
                        nc.tensor.matmul(po[:, osl], cb["li2a"][:], vt[:, re],
                                         start=(k == 0), stop=False)
                    for k in range(8):
                        osl = slice(k * HO, (k + 1) * HO)
                        im = slice(k * WV + 64, k * WV + WV)
                        nc.tensor.matmul(po[:, osl], cb["li2b"][:], vt[:, im],
                                         start=False, stop=(k == 7))
                    osb = sbo.tile([HO, 8, HO], F32, tag="osb")
                    nc.scalar.copy(osb.rearrange("a b c -> a (b c)")[:], po[:])
                    nc.sync.dma_start(
                        o_d[c0 + ch0:c0 + ch0 + 8].transpose([1, 0, 2]), osb[:])
    nc.compile()
    return nc




_CACHE = {}


def _get_runner():
    if "r" in _CACHE:
        return _CACHE["r"]
    import jax
    from jax.sharding import Mesh, PartitionSpec, NamedSharding
    from jax.experimental.shard_map import shard_map
    from concourse.bass2jax import (_bass_exec_p, install_neuronx_cc_hook,
                                    partition_id_tensor)

    install_neuronx_cc_hook()
    nc = build(1024, chunk=64)
    partition_name = nc.partition_id_tensor.name if nc.partition_id_tensor else None
    in_names, out_names, out_avals, zero_outs = [], [], [], []
    for alloc in nc.m.functions[0].allocations:
        if not isinstance(alloc, mybir.MemoryLocationSet):
            continue
        name = alloc.memorylocations[0].name
        if alloc.kind == "ExternalInput":
            if name != partition_name:
                in_names.append(name)
        elif alloc.kind == "ExternalOutput":
            out_names.append(name)
            shape = tuple(alloc.tensor_shape)
            dtype = mybir.dt.np(alloc.dtype)
            out_avals.append(jax.core.ShapedArray(shape, dtype))
            zero_outs.append(np.zeros(shape, dtype))
    n_params = len(in_names)
    n_outs = len(out_avals)
    all_in = list(in_names) + list(out_names)
    if partition_name is not None:
        all_in.append(partition_name)

    def _body(*args):
        operands = list(args)
        if partition_name is not None:
            operands.append(partition_id_tensor())
        outs = _bass_exec_p.bind(
            *operands,
            out_avals=tuple(out_avals),
            in_names=tuple(all_in),
            out_names=tuple(out_names),
            lowering_input_output_aliases=(),
            sim_require_finite=True,
            sim_require_nnan=True,
            nc=nc,
        )
        return tuple(outs)

    devices = jax.devices()[:8]
    mesh = Mesh(np.asarray(devices), ("core",))
    in_specs = (PartitionSpec("core"),) * (n_params + n_outs)
    out_specs = (PartitionSpec("core"),) * len(out_names)
    sharded = jax.jit(
        shard_map(_body, mesh=mesh, in_specs=in_specs, out_specs=out_specs,
                  check_rep=False),
        donate_argnums=tuple(range(n_params, n_params + n_outs)),
        keep_unused=True,
    )
    _CACHE["r"] = (sharded, in_names, out_names, zero_outs)
    return _CACHE["r"]


def kernel(x, template):
    """Full-input depthwise cross-correlation on 8 NeuronCores.

    x [32, 256, 64, 64] f32, template [32, 256, 16, 16] f32
    -> [32, 256, 49, 49] f32. Data-parallel over batch: 4 samples/core.
    """
    x = np.ascontiguousarray(np.asarray(x), np.float32)
    template = np.ascontiguousarray(np.asarray(template), np.float32)
    sharded, in_names, out_names, zero_outs = _get_runner()
    bases = make_bases()
    feed = {"x": x.reshape(8, 1024, N, N).reshape(8 * 1024, N, N),
            "t": template.reshape(8, 1024, HT, HT).reshape(8 * 1024, HT, HT)}
    for k, v in bases.items():
        feed[k] = np.concatenate([v] * 8, axis=0)
    concat_in = [feed[nm] for nm in in_names]
    concat_zeros = [np.zeros((8 * z.shape[0], *z.shape[1:]), z.dtype)
                    for z in zero_outs]
    out_arrs = sharded(*concat_in, *concat_zeros)
    o = np.asarray(out_arrs[out_names.index("o")])
    return o.reshape(32, 256, HO, HO)

